# revision 11
# baseline (speedup 1.0000x reference)
"""Trainium2 Bass kernel for GQA compact-cache decode attention.

Problem: nn_Attention_31585189494995 (sparse_attention, memory-bound).

  B=32 seqs, H=32 q heads, HKV=8 kv heads (G=4), DH=128,
  compact caches [65536, 8, 128] f32, per-seq live window
  [base_offsets[b], base_offsets[b]+live_counts[b]).

Sharding: tensor-parallel over the 8 kv heads - core h owns kv head h
and q heads 4h..4h+3.  Each core computes attention for all 32
sequences restricted to its head -> perfectly balanced SPMD (identical
schedule on every core, different data).

Host prep (inside kernel(), numpy):
  * scatter the new k/v token into the per-head cache slice
    (reference's store_compact_kvcache) - the updated caches are not
    returned, so this never touches the device.
  * pack ONLY the live regions, bf16:
      kP [128, Wk]  - K^T columns, seq b at columns cum[b]..cum[b]+L_b
      vP [128, Wv]  - V rows chunked by 128: seq b chunk j at columns
                      (cumnj[b]+j)*128.., partition p = cache row j*128+p
      qT [128, 128] - column 4b+g = q[b, 4h+g, :]
  * device kernel is live_counts-specialized (schedule baked at trace
    time from the actual input values - compiled per call anyway).

Device kernel (per core), all in "S^T" orientation (keys on
partitions), so every matmul writes PSUM at partition base 0:

  for each wave w (seqs split into WAVES consecutive blocks) and each
  128-key chunk j:
    st[j]  [128, 4*wave_seqs] f32 psum, memset -1e30 (mask)
    per seq b in wave: st[0:Lc, 4b..] = kP_chunk(b,j).T.T @ q[:,4b..]
       (lhsT = K^T chunk [128(DH), Lc], rhs = q [128(DH), 4])
    pT[j] = exp(st[j] * SCALE)      (no max-shift: |s*SCALE| <~ 5)
    l[1, cols] += ones.T @ pT[j]    (matmul-accumulated column sums)
    oT[:, 4b..] += V_chunk(b,j).T.T @ pT[j][:, 4b..]  (accum over j)

  outputs: oT [128, B*G] unnormalized, lT [1, B*G] sums.
  Host: o[b, 4h+g, :] = oT[:, 4b+g] / lT[4b+g].
"""

import math

import numpy as np

B = 32
H = 32
HKV = 8
G = H // HKV  # 4
DH = 128
MAX_LIVE = 2048
SCALE = DH ** -0.5
NCORES = 8
P = 128

# compute dtype for q/k/v/p on device ("bf16" or "f32")
COMPUTE_DTYPE = "bf16"

WAVES = 2  # seq blocks; waves pipeline DMA vs compute
_GROUP_TARGET_COLS = 4096  # columns of kP per DMA group (~1MB bf16)
_MASK_NEG = -1.0e30


def _apply_drain_patch():
    """The shipped walrus codegen only accepts ONE sync-wait on a Drain
    (TPB_CTRL_NO_STRUCT).  Tile's tail drain carries one wait per
    outstanding proc sem -> split them across chained drains."""
    import bass_rust
    import concourse.tile as tile
    from concourse.tile import ScopedClock

    if getattr(tile.TileContext, "_drain_patch_applied", False):
        return

    def _drain_and_barrier_split(self, tick_clock, wait_clock):
        nc = self.nc
        drain_inst = nc.sync.drain()
        wait_clock.add_sem_waits(
            drain_inst.ins, ScopedClock({None: tick_clock.global_clock})
        )
        raw = drain_inst.ins
        si = raw.sync_info
        waits = list(si.on_wait or []) if si is not None else []
        if len(waits) > 1:
            si.on_wait = waits[:1]
            for w in waits[1:]:
                extra = nc.sync.drain()
                eraw = extra.ins
                if eraw.sync_info is None:
                    eraw.sync_info = bass_rust.SyncInfo(on_wait=[w], on_update=[])
                else:
                    eraw.sync_info.on_wait = [w]

        nc.all_engine_barrier()
        assert self.sems is not None
        popped = nc._tile_sem_poison_stack.pop()
        assert popped is self._sem_poison
        nc.clear_and_free_semaphores(list(self.sems.allocated().values()))
        nc.all_engine_barrier()

    tile.TileContext._drain_and_barrier = _drain_and_barrier_split
    tile.TileContext._drain_patch_applied = True


def _split_excess_waits(nc, max_waits=1):
    """This nightly walrus codegen accepts at most ONE sync-wait per
    instruction (setupSyncWait: 'Too many sync wait commands').  Hoist
    excess waits onto single-wait NoOp carriers inserted right before
    the instruction on the same engine (per-engine program order is
    preserved, so semantics are identical)."""
    import bass_rust

    k = 0
    for fn in nc.m.functions:
        for blk in fn.blocks:
            insts = list(blk.instructions)
            out = []
            changed = False
            for inst in insts:
                si = inst.sync_info
                waits = list(si.on_wait or []) if si is not None else []
                if len(waits) > max_waits:
                    for w in waits[:-max_waits]:
                        n = bass_rust.InstNoOp(
                            name=f"wc{k}_{inst.name}", ins=[], outs=[]
                        )
                        k += 1
                        n.engine = inst.engine
                        n.sync_info = bass_rust.SyncInfo(
                            on_wait=[w], on_update=[]
                        )
                        out.append(n)
                    si.on_wait = waits[-max_waits:]
                    changed = True
                out.append(inst)
            if changed:
                blk.instructions = out


def _schedule(live):
    """Static schedule from live_counts (python ints)."""
    live = [int(x) for x in live]
    cum = [0]
    for L in live:
        cum.append(cum[-1] + L)
    nj = [(L + P - 1) // P for L in live]
    cumnj = [0]
    for n in nj:
        cumnj.append(cumnj[-1] + n)
    # waves: consecutive blocks of B/WAVES seqs
    per_wave = B // WAVES
    waves = [(w * per_wave, (w + 1) * per_wave) for w in range(WAVES)]
    # DMA groups: consecutive seqs totalling >= _GROUP_TARGET_COLS k cols,
    # never straddling a wave boundary
    groups = []
    for wb0, wb1 in waves:
        b0 = wb0
        acc = 0
        for b in range(wb0, wb1):
            acc += live[b]
            if acc >= _GROUP_TARGET_COLS or b == wb1 - 1:
                groups.append((b0, b + 1))
                b0 = b + 1
                acc = 0
    return live, cum, nj, cumnj, waves, groups


def build_program(live, reps=0, split_waits=True):
    """Build the Bass program. reps>0 wraps the body in a hardware loop
    (timing variant; body is idempotent so results stay correct)."""
    _apply_drain_patch()
    import concourse.bass as bass
    import concourse.mybir as mybir
    import concourse.tile as tile

    live, cum, nj, cumnj, waves, groups = _schedule(live)
    Wk = cum[-1]
    Wv = cumnj[-1] * P
    dt_c = mybir.dt.bfloat16 if COMPUTE_DTYPE == "bf16" else mybir.dt.float32
    f32 = mybir.dt.float32

    nc = bass.Bass(trn_type="TRN2")
    qT_d = nc.dram_tensor("qT", [P, B * G], dt_c, kind="ExternalInput")
    kP_d = nc.dram_tensor("kP", [P, Wk], dt_c, kind="ExternalInput")
    vP_d = nc.dram_tensor("vP", [P, Wv], dt_c, kind="ExternalInput")
    oT_d = nc.dram_tensor("oT", [P, B * G], f32, kind="ExternalOutput")
    lT_d = nc.dram_tensor("lT", [1, B * G], f32, kind="ExternalOutput")

    seq_group = {}
    for gi, (b0, b1) in enumerate(groups):
        for b in range(b0, b1):
            seq_group[b] = gi

    def body(tc, cpool, kvpool, stpool, opool, q_sb, ones_sb):
        o_ps = opool.tile([P, B * G], f32, tag="o")
        l_ps = opool.tile([1, B * G], f32, tag="l")

        kt = {}
        vt = {}
        for gi, (b0, b1) in enumerate(groups):
            kc0, kc1 = cum[b0], cum[b1]
            t = kvpool.tile([P, kc1 - kc0], dt_c, tag=f"k{gi}")
            nc.sync.dma_start(t[:], kP_d[:, kc0:kc1])
            kt[gi] = (t, kc0)
            vc0, vc1 = cumnj[b0] * P, cumnj[b1] * P
            tv = kvpool.tile([P, vc1 - vc0], dt_c, tag=f"v{gi}")
            nc.sync.dma_start(tv[:], vP_d[:, vc0:vc1])
            vt[gi] = (tv, vc0)

        for wi, (wb0, wb1) in enumerate(waves):
            ncols = (wb1 - wb0) * G
            col0 = wb0 * G
            njw = max(nj[b] for b in range(wb0, wb1))
            pTs = []
            for j in range(njw):
                st = stpool.tile([P, ncols], f32, tag="st")
                # mask only needed when some (seq, chunk) is dead/partial
                if any(
                    j >= nj[b] or (live[b] - j * P) < P
                    for b in range(wb0, wb1)
                ):
                    nc.vector.memset(st[:], _MASK_NEG)
                for b in range(wb0, wb1):
                    if j >= nj[b]:
                        continue
                    Lc = min(P, live[b] - j * P)
                    t, kc0 = kt[seq_group[b]]
                    kcol = cum[b] + j * P - kc0
                    nc.tensor.matmul(
                        st[0:Lc, (b - wb0) * G : (b - wb0) * G + G],
                        lhsT=t[:, kcol : kcol + Lc],
                        rhs=q_sb[:, b * G : b * G + G],
                        start=True,
                        stop=True,
                    )
                pT = cpool.tile([P, ncols], dt_c, tag=f"pT{wi}_{j}")
                nc.scalar.activation(
                    pT[:], st[:], mybir.ActivationFunctionType.Exp,
                    bias=0.0, scale=SCALE,
                )
                nc.tensor.matmul(
                    l_ps[0:1, col0 : col0 + ncols],
                    lhsT=ones_sb[:],
                    rhs=pT[:],
                    start=(j == 0),
                    stop=(j == njw - 1),
                )
                pTs.append(pT)
            # AV seq-major: PSUM accumulation groups must not interleave
            # within a bank (start=True clears has_written for the WHOLE
            # bank), so finish each seq's group before starting the next.
            for b in range(wb0, wb1):
                tv, vc0 = vt[seq_group[b]]
                for j in range(nj[b]):
                    vcol = (cumnj[b] + j) * P - vc0
                    nc.tensor.matmul(
                        o_ps[:, b * G : b * G + G],
                        lhsT=tv[:, vcol : vcol + P],
                        rhs=pTs[j][:, (b - wb0) * G : (b - wb0) * G + G],
                        start=(j == 0),
                        stop=(j == nj[b] - 1),
                    )

        o_sb = cpool.tile([P, B * G], f32, tag="osb")
        nc.vector.tensor_copy(o_sb[:], o_ps[:])
        nc.sync.dma_start(oT_d[:], o_sb[:])
        l_sb = cpool.tile([1, B * G], f32, tag="lsb")
        nc.vector.tensor_copy(l_sb[:], l_ps[:])
        nc.sync.dma_start(lT_d[:], l_sb[:])

    with tile.TileContext(nc) as tc:
        with (
            tc.tile_pool(name="const", bufs=1) as cpool,
            tc.tile_pool(name="kv", bufs=1) as kvpool,
            tc.tile_pool(name="stps", bufs=3, space="PSUM") as stpool,
            tc.tile_pool(name="ops", bufs=1, space="PSUM") as opool,
        ):
            q_sb = cpool.tile([P, B * G], dt_c, tag="q")
            nc.sync.dma_start(q_sb[:], qT_d[:])
            ones_sb = cpool.tile([P, 1], dt_c, tag="ones")
            nc.vector.memset(ones_sb[:], 1.0)
            if reps > 0:
                with tc.For_i(0, reps, 1):
                    body(tc, cpool, kvpool, stpool, opool, q_sb, ones_sb)
            else:
                body(tc, cpool, kvpool, stpool, opool, q_sb, ones_sb)
    if split_waits:
        _split_excess_waits(nc)
    return nc


def _np_cdt():
    if COMPUTE_DTYPE == "bf16":
        import ml_dtypes

        return np.dtype(ml_dtypes.bfloat16)
    return np.dtype(np.float32)


def prep_core_inputs(h, q, k, v, kc, vc, base_offsets, live_counts, slot_mapping):
    """Numpy host prep for core h. Returns {qT, kP, vP}."""
    cdt = _np_cdt()
    live, cum, nj, cumnj, _, _ = _schedule(live_counts)
    Wk = cum[-1]
    Wv = cumnj[-1] * P

    k_h = np.ascontiguousarray(kc[:, h, :])
    v_h = np.ascontiguousarray(vc[:, h, :])
    slots = np.asarray(slot_mapping)
    valid = slots >= 0
    if valid.any():
        k_h[slots[valid]] = k[valid, h, :]
        v_h[slots[valid]] = v[valid, h, :]

    kP = np.empty((P, Wk), dtype=cdt)
    vP = np.empty((P, Wv), dtype=cdt)
    for b in range(B):
        o = int(base_offsets[b])
        L = live[b]
        kP[:, cum[b] : cum[b] + L] = k_h[o : o + L].T
        nb = nj[b]
        blk = v_h[o : o + nb * P]  # [nb*128, 128]
        vP[:, cumnj[b] * P : (cumnj[b] + nb) * P] = (
            blk.reshape(nb, P, DH).transpose(1, 0, 2).reshape(P, nb * DH)
        )
    q_h = q[:, G * h : G * h + G, :]  # [B, G, DH]
    qT = np.ascontiguousarray(q_h.reshape(B * G, DH).T).astype(cdt)
    return {"qT": qT, "kP": kP, "vP": vP}


def assemble_output(core_results):
    o = np.empty((B, H, DH), dtype=np.float32)
    for h in range(NCORES):
        oT = np.asarray(core_results[h]["oT"], np.float64)
        lT = np.asarray(core_results[h]["lT"], np.float64).reshape(B * G)
        on = (oT / lT[None, :]).astype(np.float32)
        o[:, G * h : G * h + G, :] = on.T.reshape(B, G, DH)
    return o


def kernel(
    q,
    k,
    v,
    compact_k_cache,
    compact_v_cache,
    base_offsets,
    live_counts,
    slot_mapping,
):
    from concourse.bass_utils import run_bass_kernel_spmd

    q = np.asarray(q, dtype=np.float32)
    k = np.asarray(k, dtype=np.float32)
    v = np.asarray(v, dtype=np.float32)
    kc = np.asarray(compact_k_cache, dtype=np.float32)
    vc = np.asarray(compact_v_cache, dtype=np.float32)
    base_offsets = np.asarray(base_offsets)
    live_counts = np.asarray(live_counts)
    slot_mapping = np.asarray(slot_mapping)

    nc = build_program(live_counts)
    in_maps = [
        prep_core_inputs(
            h, q, k, v, kc, vc, base_offsets, live_counts, slot_mapping
        )
        for h in range(NCORES)
    ]
    res = run_bass_kernel_spmd(nc, in_maps, core_ids=list(range(NCORES)))
    return assemble_output(res.results)
